# revision 4
# baseline (speedup 1.0000x reference)
"""Causal attention kernel for 8 Trainium2 NeuronCores (v3, mixed fp8).

Problem: x[4,4096,1024] @ {Wq,Wk,Wv}[1024,64] (+bias) -> causal attention
with softmax scaled by sqrt(seq)=64 -> out[4,4096,64].

Sharding: 8 cores = (batch b in 0..3) x (half h in 0..1). Queries are
interleaved at 256-row stripe granularity: core (b, h) owns query stripes
{512i+256h : +256} for i in 0..7, so the causal key extent per stripe is
identical on every core. Keys/values cover the full 4096-key batch.

v3 key changes vs v2 (105.5us baseline):
  - fp8e4 DoubleRow matmuls (2 contraction subtiles per instruction at
    0.5 cyc/col) for the bulk of the PE work:
      * projections of row groups 1-3,5-7 (x pre-cast to fp8 on host;
        groups 0,4 = global rows 0-1023 stay bf16 so early-sequence
        queries, which average few keys, see accurate K/V),
      * attention AV for non-diagonal key pairs (exp output et and v
        both fp8; the diagonal pair stays bf16 because early queries
        read those v values with weight ~1).
    Scores stay bf16. Numpy-validated rel err 3.4e-3 (budget 2e-2).
  - v bias is folded into the vstage copy (tensor_scalar_add), and bv
    dropped from the epilogue (softmax rows sum to 1).
  - output accumulates in a persistent SBUF tile; 4 batched [128,256]
    DMAs (after stripes 1,3,5,7) replace 16 tail stores.
  - x DMA drops from 8 MiB to 5 MiB/core (fp8 groups).

Layouts per core:
  xtb/xt8: groups of [128, 4096]; row 128g+p, col 512c+r = x^T[d_in=
      128c+p, local row 512g+r]. Local rows: groups 0-3 = own stripes
      (8 stripes of 256 in order), groups 4-7 = partner-half stripes.
      Groups 0,4 bf16 (xtb), groups 1-3,5-7 fp8 (xt8).
  kTd/qTd: [64, S|NB] bf16.
  vsb8: [128, 32*66] fp8, per key tile a [128,65] v_aug block (v|ones).
  vsbb: [128, 16*66] bf16, same for own tiles only (diagonal AV).
  out_sb: [128, 16*64] fp32; col 64*(2i+t)+o, row p = out row 256i+128t+p.
"""

import sys

sys.path.insert(0, "/opt/trn_rl_repo")

from contextlib import ExitStack

import ml_dtypes
import numpy as np

import concourse.bacc as bacc
import concourse.mybir as mybir
import concourse.tile as tile
from concourse.bass import ds, ts
from concourse.bass_utils import run_bass_kernel_spmd
from concourse.masks import make_identity

B, S, D_IN, D_OUT = 4, 4096, 1024, 64
NB = S // 2  # 2048 query rows per core
N_CORES = 8
NEG = -100.0  # additive pre-exp mask value; exp(-100+s) flushes to 0
SCALE = 1.0 / 64.0  # 1/sqrt(seq)

FP32 = mybir.dt.float32
BF16 = mybir.dt.bfloat16
F8 = mybir.dt.float8e4
DR = mybir.MatmulPerfMode.DoubleRow

N_KT = S // 128  # 32 key tiles of 128
VW = 66  # v_aug block stride, bf16 diag copy (64 v + ones + pad)
VW8 = 80  # v_aug block stride, fp8 copy (DoubleRow LDW needs stride%16==0)
QW = 2  # key-tile pairs per fp8 exp job (psum tile = QW banks)

BF_GROUPS = (0, 4)  # row groups projected in bf16 (global rows 0-1023)
F8_GROUPS = (1, 2, 3, 5, 6, 7)


def build_program():
    nc = bacc.Bacc("TRN2", target_bir_lowering=False, debug=False)

    xtb = nc.declare_dram_parameter("xtb", [2 * 128, S], BF16, isOutput=False)
    xt8 = nc.declare_dram_parameter("xt8", [6 * 128, S], F8, isOutput=False)
    wkvb = nc.declare_dram_parameter("wkvb", [D_IN, 128], BF16, isOutput=False)
    wqb = nc.declare_dram_parameter("wqb", [D_IN, 64], BF16, isOutput=False)
    wkv8 = nc.declare_dram_parameter("wkv8", [D_IN, 128], F8, isOutput=False)
    wq8 = nc.declare_dram_parameter("wq8", [D_IN, 64], F8, isOutput=False)
    bqk = nc.declare_dram_parameter("bqk", [64, 2], FP32, isOutput=False)
    bv_r = nc.declare_dram_parameter("bv_r", [64, 1], FP32, isOutput=False)
    hbias = nc.declare_dram_parameter("hbias", [128, 1], FP32, isOutput=False)
    out = nc.declare_dram_parameter("out", [128, 16 * 64], FP32, isOutput=True)

    with tile.TileContext(nc) as tc, ExitStack() as ctx:
        const = ctx.enter_context(tc.tile_pool(name="const", bufs=1))
        xin = ctx.enter_context(tc.tile_pool(name="xin", bufs=1))
        pers = ctx.enter_context(tc.tile_pool(name="pers", bufs=1))
        vst = ctx.enter_context(tc.tile_pool(name="vst", bufs=2))
        expb = ctx.enter_context(tc.tile_pool(name="expb", bufs=2))
        exp8 = ctx.enter_context(tc.tile_pool(name="exp8", bufs=4))
        avsb = ctx.enter_context(tc.tile_pool(name="avsb", bufs=2))
        prtp = ctx.enter_context(tc.tile_pool(name="prtp", bufs=1))
        outp = ctx.enter_context(tc.tile_pool(name="outp", bufs=4))
        ps_kv = ctx.enter_context(tc.tile_pool(name="ps_kv", bufs=1, space="PSUM"))
        ps_s = ctx.enter_context(tc.tile_pool(name="ps_s", bufs=2, space="PSUM"))
        ps_sm = ctx.enter_context(tc.tile_pool(name="ps_sm", bufs=2, space="PSUM"))
        ps_av = ctx.enter_context(tc.tile_pool(name="ps_av", bufs=1, space="PSUM"))

        # --- constants -----------------------------------------------------
        ident = const.tile([128, 128], FP32)
        make_identity(nc, ident[:])
        identb = const.tile([64, 64], BF16)
        make_identity(nc, identb[:])

        # diagonal mask for a key-tile PAIR laid out side by side in one
        # [128, 512] tile: half j covers key tile 2i+j of stripe i.
        # mask[p, 256j+f] = 0 where f >= 128j+p else NEG/SCALE (pre-exp-scale)
        mask2 = const.tile([128, 512], FP32)
        nc.gpsimd.memset(mask2[:], 0.0)
        for j in range(2):
            nc.gpsimd.affine_select(
                out=mask2[:, ds(256 * j, 256)],
                in_=mask2[:, ds(256 * j, 256)],
                compare_op=mybir.AluOpType.is_ge,
                fill=NEG / SCALE,
                base=-128 * j,
                pattern=[[1, 256]],
                channel_multiplier=-1,
            )

        wkv_sb = const.tile([128, 8 * 128], BF16)
        wq_sb = const.tile([128, 8 * 64], BF16)
        wkv8_sb = const.tile([128, 8 * 128], F8)
        wq8_sb = const.tile([128, 8 * 64], F8)
        bqk_sb = const.tile([64, 2], FP32)
        bv_sb = const.tile([64, 1], FP32)
        hb_sb = const.tile([128, 1], FP32)

        # --- persistent intermediates -------------------------------------
        kTd = pers.tile([64, S], BF16)
        qTd = pers.tile([64, NB], BF16)
        vsb8 = pers.tile([128, N_KT * VW8], F8)  # v_aug per ktile (all)
        vsbb = pers.tile([128, 16 * VW], BF16)  # v_aug own ktiles (diag)
        out_sb = pers.tile([128, 16 * 64], FP32)
        ones1 = const.tile([128, 1], FP32)
        nc.vector.memset(ones1[:], 1.0)
        v8_ones = vsb8[:].rearrange("p (t c) -> p t c", c=VW8)[:, :, 64:65]
        nc.vector.tensor_copy(out=v8_ones, in_=ones1[:].broadcast_to([128, N_KT, 1]))
        vb_ones = vsbb[:].rearrange("p (t c) -> p t c", c=VW)[:, :, 64:65]
        nc.vector.tensor_copy(out=vb_ones, in_=ones1[:].broadcast_to([128, 16, 1]))

        # --- phase 1: projections for one 512-row group -------------------
        xg_tiles = [None] * 8

        def dma_group(g):
            if g in BF_GROUPS:
                r0 = 128 * BF_GROUPS.index(g)
                halves = []
                for hh in range(2):
                    xh = xin.tile([128, 2048], BF16, tag=f"xgb{g}{hh}")
                    nc.sync.dma_start(xh[:], xtb[ds(r0, 128), ds(2048 * hh, 2048)])
                    halves.append(xh)
                xg_tiles[g] = halves
            else:
                r0 = 128 * F8_GROUPS.index(g)
                xh = xin.tile([128, S], F8, tag=f"xg8{g}")
                nc.sync.dma_start(xh[:], xt8[ds(r0, 128), :])
                xg_tiles[g] = xh

        def row_group(g):
            """g in 0..7; 0-3 own rows (with q), 4-7 partner rows."""
            has_q = g < 4
            r0 = 512 * g if has_q else 512 * (g - 4)
            pkv = ps_kv.tile([128, 512], FP32, tag="ps_kv")
            if has_q:
                pq = ps_av.tile([64, 512], FP32, tag="ps_av")
            else:
                pq = None
            if g in BF_GROUPS:
                xg = xg_tiles[g]
                for c in range(8):
                    xh = xg[c // 4][:, ts(c % 4, 512)]
                    nc.tensor.matmul(
                        pkv[:],
                        wkv_sb[:, ts(c, 128)],
                        xh,
                        start=(c == 0),
                        stop=(c == 7),
                    )
                    if has_q:
                        nc.tensor.matmul(
                            pq[:],
                            wq_sb[:, ts(c, 64)],
                            xh,
                            start=(c == 0),
                            stop=(c == 7),
                        )
            else:
                xg = xg_tiles[g]
                for cc in range(4):
                    xh2 = xg[:, ds(1024 * cc, 1024)].rearrange(
                        "p (t r) -> p t r", t=2
                    )
                    nc.tensor.matmul(
                        pkv[:],
                        wkv8_sb[:, ds(256 * cc, 256)].rearrange(
                            "p (t k) -> p t k", t=2
                        ),
                        xh2,
                        start=(cc == 0),
                        stop=(cc == 3),
                        perf_mode=DR,
                    )
                    if has_q:
                        nc.tensor.matmul(
                            pq[:],
                            wq8_sb[:, ds(128 * cc, 128)].rearrange(
                                "p (t k) -> p t k", t=2
                            ),
                            xh2,
                            start=(cc == 0),
                            stop=(cc == 3),
                            perf_mode=DR,
                        )
            # column base in kTd/vsb: own rows -> tiles 0-15, partner -> 16-31
            k0 = 512 * g if has_q else 2048 + 512 * (g - 4)
            if has_q:
                nc.vector.tensor_scalar_add(
                    out=qTd[:, ds(r0, 512)],
                    in0=pq[:],
                    scalar1=bqk_sb[:, 0:1],
                )
            nc.vector.tensor_scalar_add(
                out=kTd[:, ds(k0, 512)],
                in0=pkv[0:64, :],
                scalar1=bqk_sb[:, 1:2],
            )
            # v: bias-add + transpose [64,512] psum slice to [512 rows, 64]
            vstage = vst.tile([64, 512], BF16, tag="vstage")
            nc.vector.tensor_scalar_add(
                out=vstage[:], in0=pkv[64:128, :], scalar1=bv_sb[:, 0:1]
            )
            psv = ps_sm.tile([128, 4 * VW], BF16, tag="ps_sm")
            for t in range(4):
                nc.tensor.matmul(
                    psv[:, ds(VW * t, 64)],
                    vstage[:, ts(t, 128)],
                    identb[:],
                    start=(t == 0),
                    stop=(t == 3),
                    is_transpose=True,
                )
            kt0 = k0 // 128
            vsrc = psv[:].rearrange("p (t c) -> p t c", c=VW)[:, :, 0:64]
            v8dst = vsb8[:, ds(VW8 * kt0, 4 * VW8)].rearrange(
                "p (t c) -> p t c", c=VW8
            )[:, :, 0:64]
            nc.vector.tensor_copy(out=v8dst, in_=vsrc)
            if has_q:
                vbdst = vsbb[:, ds(VW * kt0, 4 * VW)].rearrange(
                    "p (t c) -> p t c", c=VW
                )[:, :, 0:64]
                nc.vector.tensor_copy(out=vbdst, in_=vsrc)

        # --- phase 2: attention for one 256-query stripe -------------------
        # Jobs per span: optional diag (own pair i, bf16 et/v, masked),
        # then QW-pair fp8 jobs (each pair = one DoubleRow AV matmul),
        # then (final only) the boundary partner pair (fp8, hbias exp).
        # Software-pipelined: scores of job n+1 are emitted before the AV
        # matmuls of job n.
        partials = {}

        def stripe_span(i, l_lo, l_hi, o_lo, o_hi, final):
            q_lo = qTd[:, ds(256 * i, 256)]
            pav = ps_av.tile([65, 256], FP32, tag="ps_av")
            own_nd = [p for p in range(l_lo, min(l_hi, i + 1)) if p != i]
            oth = list(range(o_lo, min(o_hi, i)))
            old_kt0s = [2 * p for p in own_nd] + [16 + 2 * p for p in oth]
            jobs = []
            if l_lo <= i < l_hi:
                jobs.append(("diag", [2 * i]))
            for n in range(0, len(old_kt0s) - 1, 2):
                jobs.append(("old", old_kt0s[n : n + 2]))
            if len(old_kt0s) % 2:
                jobs.append(("old", old_kt0s[-1:]))
            if final:
                jobs.append(("bnd", [16 + 2 * i]))

            def scores(kind, kt0s):
                psc = ps_s.tile([128, QW * 512], FP32, tag="ps_s")
                for idx, kt0 in enumerate(kt0s):
                    for j in range(2):
                        nc.tensor.matmul(
                            psc[:, ds(512 * idx + 256 * j, 256)],
                            kTd[:, ts(kt0 + j, 128)],
                            q_lo,
                            start=(j == 0),
                            stop=(j == 1),
                        )
                if kind == "diag":
                    nc.vector.tensor_add(
                        psc[:, 0:512], psc[:, 0:512], mask2[:]
                    )
                return psc

            def expo(kind, psc, kt0s):
                w = 512 * len(kt0s)
                if kind == "diag":
                    et = expb.tile([128, 512], BF16, tag="etb")
                else:
                    et = exp8.tile([128, QW * 512], F8, tag="et8")
                nc.scalar.activation(
                    et[:, 0:w],
                    psc[:, 0:w],
                    mybir.ActivationFunctionType.Exp,
                    bias=hb_sb[:, 0:1] if kind == "bnd" else 0.0,
                    scale=SCALE,
                )
                return et

            def av(kind, et, kt0s, first, last):
                if kind == "diag":
                    kt0 = kt0s[0]
                    for j in range(2):
                        nc.tensor.matmul(
                            pav[:],
                            vsbb[:, ds(VW * (kt0 + j), 65)],
                            et[:, ds(256 * j, 256)],
                            start=(first and j == 0),
                            stop=(last and j == 1),
                        )
                else:
                    for idx, kt0 in enumerate(kt0s):
                        v2 = vsb8[:, ds(VW8 * kt0, 2 * VW8)].rearrange(
                            "p (t c) -> p t c", t=2
                        )[:, :, 0:65]
                        e2 = et[:, ds(512 * idx, 512)].rearrange(
                            "p (t r) -> p t r", t=2
                        )
                        nc.tensor.matmul(
                            pav[:],
                            v2,
                            e2,
                            start=(first and idx == 0),
                            stop=(last and idx == len(kt0s) - 1),
                            perf_mode=DR,
                        )

            prev = None
            for n, (kind, kt0s) in enumerate(jobs):
                psc = scores(kind, kt0s)
                et = expo(kind, psc, kt0s)
                if prev is not None:
                    av(prev[0], prev[1], prev[2], prev[3], False)
                prev = (kind, et, kt0s, n == 0)
            av(prev[0], prev[1], prev[2], prev[3], True)

            if not final:
                if i in partials:
                    nc.vector.tensor_add(partials[i][:], partials[i][:], pav[:])
                else:
                    part = prtp.tile([65, 256], FP32, tag=f"part{i}")
                    nc.vector.tensor_copy(out=part[:], in_=pav[:])
                    partials[i] = part
                return

            # epilogue: transpose av back to [q, 65], normalize
            av_sb = avsb.tile([66, 256], FP32, tag="av")
            if i in partials:
                nc.vector.tensor_add(av_sb[0:65, :], pav[:], partials.pop(i)[:])
            else:
                nc.vector.tensor_copy(out=av_sb[0:65, :], in_=pav[:])
            pso = ps_sm.tile([128, 2 * VW], FP32, tag="ps_sm")
            for t in range(2):
                nc.tensor.matmul(
                    pso[:, ds(VW * t, 66)],
                    av_sb[:, ts(t, 128)],
                    ident[0:66, 0:66],
                    start=(t == 0),
                    stop=(t == 1),
                    is_transpose=True,
                )
            rec = outp.tile([128, 2], FP32, tag="rec")
            for t in range(2):
                nc.vector.reciprocal(rec[:, ds(t, 1)], pso[:, ds(VW * t + 64, 1)])
                nc.vector.tensor_scalar_mul(
                    out=out_sb[:, ds(64 * (2 * i + t), 64)],
                    in0=pso[:, ds(VW * t, 64)],
                    scalar1=rec[:, ds(t, 1)],
                )
            if i % 2 == 1:
                k = i // 2
                nc.sync.dma_start(
                    out[:, ds(256 * k, 256)], out_sb[:, ds(256 * k, 256)]
                )

        def stripe(i):
            stripe_span(i, 0, 8, 0, 8, True)

        # schedule: DMAs upfront in dependency order; projections as early
        # as possible, attention stripes as soon as their key tiles +
        # queries are ready.
        nc.sync.dma_start(
            wkv_sb[:].rearrange("p (c o) -> p c o", c=8),
            wkvb[:, :].rearrange("(c p) o -> p c o", p=128),
        )
        xh00 = xin.tile([128, 2048], BF16, tag="xgb00")
        nc.sync.dma_start(xh00[:], xtb[ds(0, 128), ds(0, 2048)])
        nc.sync.dma_start(
            wq_sb[:].rearrange("p (c o) -> p c o", c=8),
            wqb[:, :].rearrange("(c p) o -> p c o", p=128),
        )
        xh01 = xin.tile([128, 2048], BF16, tag="xgb01")
        nc.sync.dma_start(xh01[:], xtb[ds(0, 128), ds(2048, 2048)])
        xg_tiles[0] = [xh00, xh01]
        nc.sync.dma_start(bqk_sb[:], bqk[:, :])
        nc.sync.dma_start(bv_sb[:], bv_r[:, :])
        nc.sync.dma_start(hb_sb[:], hbias[:, :])
        dma_group(4)
        nc.sync.dma_start(
            wkv8_sb[:].rearrange("p (c o) -> p c o", c=8),
            wkv8[:, :].rearrange("(c p) o -> p c o", p=128),
        )
        nc.sync.dma_start(
            wq8_sb[:].rearrange("p (c o) -> p c o", c=8),
            wq8[:, :].rearrange("(c p) o -> p c o", p=128),
        )
        for g in (1, 5, 2, 6, 3, 7):
            dma_group(g)

        row_group(0)
        stripe_span(0, 0, 1, 0, 0, False)
        stripe_span(1, 0, 2, 0, 0, False)
        row_group(4)
        stripe_span(0, 1, 1, 0, 0, True)
        stripe_span(1, 2, 2, 0, 2, True)
        row_group(1)
        row_group(5)
        stripe(2)
        stripe(3)
        row_group(2)
        stripe_span(5, 0, 3, 0, 3, False)
        row_group(6)
        stripe(4)
        row_group(3)
        stripe_span(6, 0, 5, 0, 5, False)
        stripe_span(7, 0, 5, 0, 5, False)
        stripe_span(5, 3, 8, 3, 8, True)
        row_group(7)
        stripe_span(6, 5, 8, 5, 8, True)
        stripe_span(7, 5, 8, 5, 8, True)

    return nc


_program = None


def _get_program():
    global _program
    if _program is None:
        _program = build_program()
        _program.finalize()
    return _program


def build_in_maps(x, Wq, bq, Wk, bk, Wv, bv):
    x = np.asarray(x, dtype=np.float32)
    Wq = np.asarray(Wq, dtype=np.float32)
    bq = np.asarray(bq, dtype=np.float32)
    Wk = np.asarray(Wk, dtype=np.float32)
    bk = np.asarray(bk, dtype=np.float32)
    Wv = np.asarray(Wv, dtype=np.float32)
    bv = np.asarray(bv, dtype=np.float32)

    bf = ml_dtypes.bfloat16
    f8 = ml_dtypes.float8_e4m3
    wkv_np = np.ascontiguousarray(np.concatenate([Wk, Wv], axis=1))
    wq_np = np.ascontiguousarray(Wq)
    bqk_np = np.ascontiguousarray(np.stack([bq, bk], axis=1))  # [64, 2]
    bv_np = np.ascontiguousarray(bv[:, None])  # [64, 1]

    in_maps = []
    for c in range(N_CORES):
        b, h = c // 2, c % 2
        # local row order: own stripes 0-7 then partner stripes 0-7
        loc = [x[b, 512 * i + 256 * h : 512 * i + 256 * h + 256] for i in range(8)]
        oth = [
            x[b, 512 * i + 256 * (1 - h) : 512 * i + 256 * (1 - h) + 256]
            for i in range(8)
        ]
        x_lc = np.concatenate(loc + oth, axis=0)  # [4096, 1024]
        # group g -> xt[128, 4096]: xt[p, 512c+r] = x_lc[512g+r, 128c+p]
        xt_all = x_lc.reshape(8, 512, 8, 128).transpose(0, 3, 2, 1)
        xtb_np = np.ascontiguousarray(
            xt_all[list(BF_GROUPS)].reshape(2 * 128, S)
        ).astype(bf)
        xt8_np = np.ascontiguousarray(
            xt_all[list(F8_GROUPS)].reshape(6 * 128, S)
        ).astype(f8)
        # cross-half boundary bias: stripe i's partner pair p==i is in the
        # past for h=1 (valid) and in the future for h=0 (masked)
        hb = np.full((128, 1), 0.0 if h == 1 else NEG, np.float32)
        in_maps.append(
            {
                "xtb": xtb_np,
                "xt8": xt8_np,
                "wkvb": wkv_np.astype(bf),
                "wqb": wq_np.astype(bf),
                "wkv8": wkv_np.astype(f8),
                "wq8": wq_np.astype(f8),
                "bqk": bqk_np,
                "bv_r": bv_np,
                "hbias": hb,
            }
        )
    return in_maps


def unshard_out(results):
    out_full = np.empty((B, S, D_OUT), np.float32)
    for c in range(N_CORES):
        b, h = c // 2, c % 2
        o = np.asarray(results[c]["out"], np.float32)  # [128, 16*64]
        tmp = o.reshape(128, 8, 2, 64).transpose(1, 2, 0, 3)  # [i, t, p, 64]
        out_full[b].reshape(8, 2, 2, 128, 64)[:, h] = tmp
    return out_full


def kernel(x, Wq, bq, Wk, bk, Wv, bv):
    in_maps = build_in_maps(x, Wq, bq, Wk, bk, Wv, bv)
    nc = _get_program()
    res = run_bass_kernel_spmd(nc, in_maps, list(range(N_CORES)))
    return unshard_out(res.results)


if __name__ == "__main__":
    rng = np.random.default_rng(0)
    inputs = {
        "x": rng.standard_normal((B, S, D_IN), dtype=np.float32),
        "Wq": rng.standard_normal((D_IN, D_OUT), dtype=np.float32) * 0.02,
        "bq": rng.standard_normal(D_OUT, dtype=np.float32) * 0.02,
        "Wk": rng.standard_normal((D_IN, D_OUT), dtype=np.float32) * 0.02,
        "bk": rng.standard_normal(D_OUT, dtype=np.float32) * 0.02,
        "Wv": rng.standard_normal((D_IN, D_OUT), dtype=np.float32) * 0.02,
        "bv": rng.standard_normal(D_OUT, dtype=np.float32) * 0.02,
    }
    o = kernel(**inputs)
    print("kernel output", o.shape, o.dtype, float(np.abs(o).max()))
